# revision 1
# baseline (speedup 1.0000x reference)
"""Size-weighted focal loss on 8 Trainium2 NeuronCores — v4.

Math (per element, x = logit, t in {0,1}):
  w  = x*(1-2t)          so (1-pt) = sigmoid(w)
  N  = softplus(-w)      = ln(1 + e^{-w})
  L  = -log(pt) = softplus(w) = N + w
  s2 = sigmoid(w)^2      = e^{-2N}
  a  = 0.75 - 0.5*t      (alpha_t with ALPHA=0.25)
  elem = a * s2 * L

Device formulation (per core, 8 samples in groups [1,1,2,2,1,1] —
singles at the ends for pipeline fill/drain, pairs in the middle for ACT
pass amortization — bf16 intermediates, software-pipelined emission):
  w   = bf16 sign-flip of x in one DVE op: hi16(x) XOR (lo16(t) << 15)
  E   = exp(-w)                                  [ACT]
  N   = ln(E + 1)                                [ACT]
  s2p = exp(-2N + ln 0.5) = 0.5*s2               [ACT]
  F'  = (t - 1.5) * s2p = -(a*s2)                [DVE stt, i32 x bf16]
  PE:  per sample, psum[128,256] += F'^T @ [N | w]  (16 chunks)
  diag extract with mask M[i,i] = M[i,128+i] = -1:
    Scol[:,b] = sum(a*s2*(N+w)) partials per partition-slot
  All activations resolve to natural_log_exp_and_others (see
  _patch_act_tables) -> one ACT_TABLE_LOAD per kernel.

Host: fg_b = count_nonzero(target_b); mean_b( (S_b/HW) * sw(fg_b) ).
"""

import numpy as np
from contextlib import ExitStack

P = 128
B_PER_CORE = 8
GRP = 2                    # samples per tile group
NGRP = B_PER_CORE // GRP
N_CORES = 8
H = 512
W = 512
HW = H * W                 # 262144
FREE = HW // P             # 2048 per sample
GFREE = FREE * GRP         # free dim of a group tile
NCHUNK = FREE // P         # 16 chunks per sample
LN_HALF = -0.6931471805599453
# Schraudolph fast-exp: bitcast_f32(round(A*y + B)) ~= e^y ;
# here y = -2N + ln(0.5):  i = N*SCH_A + SCH_B
SCH_A = -24204406.323122256
SCH_B = 1056478197.0
TRICK_SINGLES = False

_GLOBAL = {}


def _patch_act_tables():
    """Steer every Exp/Ln activation to the one table set containing both
    (natural_log_exp_and_others), instead of the greedy first-match which
    alternates exp_and_others/natural_log and reloads tables per sample.
    Set order/indices are preserved; only membership is masked."""
    import concourse.bacc as bacc_mod
    import concourse.mybir as mybir
    from concourse.hw_specs import get_activation_tables as _orig

    def _patched(arch):
        A = mybir.ActivationFunctionType
        out = {}
        for name, fns in _orig(arch).items():
            if name != "natural_log_exp_and_others":
                fns = fns - {A.Exp, A.Ln}
            out[name] = fns
        return out

    bacc_mod.get_activation_tables = _patched


def _build():
    import concourse.bacc as bacc
    import concourse.tile as tile
    import concourse.mybir as mybir

    _patch_act_tables()

    f32 = mybir.dt.float32
    i32 = mybir.dt.int32
    bf16 = mybir.dt.bfloat16
    u16 = mybir.dt.uint16
    Alu = mybir.AluOpType
    Act = mybir.ActivationFunctionType

    nc = bacc.Bacc("TRN2", target_bir_lowering=False, debug=False,
                   num_devices=N_CORES)

    pred_in = nc.dram_tensor("pred", (B_PER_CORE, H, W), f32, kind="ExternalInput")
    targ_in = nc.dram_tensor("target", (B_PER_CORE, H, W), i32, kind="ExternalInput")
    mask_in = nc.dram_tensor("mask", (P, 2 * P), f32, kind="ExternalInput")
    out_t = nc.dram_tensor("out", (P, B_PER_CORE), f32, kind="ExternalOutput")

    # [b, 512, 512] -> [b, 128, 2048]; partition p holds contiguous 2048 elems
    x_v = pred_in.ap().rearrange("b (p q) w -> b p (q w)", p=P)
    t_v = targ_in.ap().rearrange("b (p q) w -> b p (q w)", p=P)

    with ExitStack() as ctx:
        tc = ctx.enter_context(tile.TileContext(nc))
        singles = ctx.enter_context(tc.tile_pool(name="singles", bufs=1))
        io = ctx.enter_context(tc.tile_pool(name="io", bufs=3))
        work = ctx.enter_context(tc.tile_pool(name="work", bufs=2))
        nwpool = ctx.enter_context(tc.tile_pool(name="nwpool", bufs=3))
        psum = ctx.enter_context(tc.tile_pool(name="psum", bufs=3, space="PSUM"))

        mask_t = singles.tile([P, 2 * P], f32)
        shift15_t = singles.tile([P, 1], u16)
        nc.vector.memset(shift15_t[:], 15)
        lnhalf_t = singles.tile([P, 1], f32)
        nc.vector.memset(lnhalf_t[:], LN_HALF)
        Scol = singles.tile([P, B_PER_CORE], f32)   # per-partition loss partials

        # Variable group sizes: singles at the ends for fast pipeline
        # fill/drain, pairs in the middle for ACT pass amortization. Tiles
        # are pair-sized; singles use the left half. Emission is software-
        # pipelined so DVE starts w16(g+1) while ACT runs group g's chain.
        groups = [(0, 1), (1, 1), (2, 2), (4, 2), (6, 1), (7, 1)]
        NG = len(groups)
        st = [dict() for _ in range(NG)]

        def emit_load(g):
            b0, gsz = groups[g]
            xt = io.tile([P, GFREE], f32, tag="xt")
            tt = io.tile([P, GFREE], i32, tag="tt")
            for k in range(gsz):
                sl = slice(k * FREE, (k + 1) * FREE)
                nc.sync.dma_start(out=xt[:, sl], in_=x_v[b0 + k])
                nc.sync.dma_start(out=tt[:, sl], in_=t_v[b0 + k])
            st[g]["xt"], st[g]["tt"] = xt, tt

        def emit_w(g):
            _, gsz = groups[g]
            gf = gsz * FREE
            xt, tt = st[g]["xt"], st[g]["tt"]
            # nw holds both PE rhs blocks: [:,0,:] = N, [:,1,:] = w
            nw = nwpool.tile([P, 2, GFREE], bf16, tag="nw")
            # w = bf16(x), sign flipped where t==1, in one DVE op:
            # (lo16(t) << 15) XOR hi16(x). u16 views keep the ALU integer.
            x_hi = xt[:, :gf].bitcast(u16).rearrange(
                "p (q two) -> p q two", two=2)[:, :, 1]
            t_lo = tt[:, :gf].bitcast(u16).rearrange(
                "p (q two) -> p q two", two=2)[:, :, 0]
            w_u16 = nw[:, 1, :gf].bitcast(u16)
            # Per-sample ops so each starts as soon as its data lands.
            winsts = []
            for lo in range(0, gf, FREE):
                sl = slice(lo, lo + FREE)
                winsts.append(nc.vector.scalar_tensor_tensor(
                    out=w_u16[:, sl], in0=t_lo[:, sl], scalar=shift15_t[:],
                    in1=x_hi[:, sl],
                    op0=Alu.logical_shift_left, op1=Alu.bitwise_xor))
            st[g]["nw"] = nw
            st[g]["winst"] = winsts[0]

        def emit_act(g):
            _, gsz = groups[g]
            gf = gsz * FREE
            nw = st[g]["nw"]
            # eb holds E = exp(-w) first, then is overwritten with
            # s2p = 0.5*s2 (E is dead once N is computed).
            eb = work.tile([P, GFREE], bf16, tag="eb")
            nc.scalar.activation(eb[:, :gf], nw[:, 1, :gf], Act.Exp,
                                 scale=-1.0)
            nc.scalar.activation(nw[:, 0, :gf], eb[:, :gf], Act.Ln, bias=1.0)
            if TRICK_SINGLES and gsz == 1 and groups[g][0] in (0, 7):
                # Single groups (fill/drain): skip ACT pass 3; s2p comes
                # from a Schraudolph fast-exp on DVE in emit_tail.
                st[g]["s2p"] = None
                return
            # s2p goes to its own small tile so eb (E) frees right after
            # the Ln pass instead of after F' (shorter WAR chain).
            s2a = work.tile([P, FREE], bf16, tag="s2a")
            if g == NG - 1:
                # Tail group: halve the last pass so F'/matmuls overlap
                h = gf // 2
                nc.scalar.activation(s2a[:, :h], nw[:, 0, :h], Act.Exp,
                                     scale=-2.0, bias=lnhalf_t[:])
                nc.scalar.activation(s2a[:, h:gf], nw[:, 0, h:gf], Act.Exp,
                                     scale=-2.0, bias=lnhalf_t[:])
            elif gsz == 2:
                # Pairs: ACT does sample 0's s2p; sample 1's comes from a
                # Schraudolph fast-exp on DVE (emit_tail) to rebalance
                # ACT vs DVE load.
                nc.scalar.activation(s2a[:, :FREE], nw[:, 0, :FREE], Act.Exp,
                                     scale=-2.0, bias=lnhalf_t[:])
                st[g]["s2half"] = True
            else:
                nc.scalar.activation(s2a[:, :gf], nw[:, 0, :gf], Act.Exp,
                                     scale=-2.0, bias=lnhalf_t[:])
            st[g]["s2p"] = s2a

        def emit_tail(g):
            b0, gsz = groups[g]
            gf = gsz * FREE
            nw, s2p, tt = st[g]["nw"], st[g]["s2p"], st[g]["tt"]
            if s2p is None:
                # s2p = 0.5*exp(-2N) via fast-exp bit trick (2x_2p ts)
                s2i = work.tile([P, FREE], i32, tag="s2i")
                nc.vector.tensor_scalar(s2i[:], nw[:, 0, :gf], SCH_A, SCH_B,
                                        Alu.mult, Alu.add)
                s2ap = s2i[:].bitcast(f32)
            else:
                s2ap = s2p[:, :min(gf, FREE)]
            # F' = (t - 1.5) * 0.5*s2 = -(0.75-0.5t)*s2 = -a*s2
            Ft = work.tile([P, GFREE], bf16, tag="Ft")
            if st[g].get("s2half"):
                # sample 1's s2p = 0.5*exp(-2N) via fast-exp bit trick
                s2i = work.tile([P, FREE], i32, tag="s2i")
                nc.vector.tensor_scalar(s2i[:], nw[:, 0, FREE:2 * FREE],
                                        SCH_A, SCH_B, Alu.mult, Alu.add)
                finst = nc.vector.scalar_tensor_tensor(
                    out=Ft[:, :FREE], in0=tt[:, :FREE], scalar=1.5,
                    in1=s2p[:, :FREE], op0=Alu.subtract, op1=Alu.mult)
                nc.vector.scalar_tensor_tensor(
                    out=Ft[:, FREE:2 * FREE], in0=tt[:, FREE:2 * FREE],
                    scalar=1.5, in1=s2i[:].bitcast(f32),
                    op0=Alu.subtract, op1=Alu.mult)
            elif g == NG - 1:
                h = gf // 2
                nc.vector.scalar_tensor_tensor(
                    out=Ft[:, :h], in0=tt[:, :h], scalar=1.5,
                    in1=s2ap[:, :h], op0=Alu.subtract, op1=Alu.mult)
                finst = nc.vector.scalar_tensor_tensor(
                    out=Ft[:, h:gf], in0=tt[:, h:gf], scalar=1.5,
                    in1=s2ap[:, h:gf], op0=Alu.subtract, op1=Alu.mult)
            else:
                finst = nc.vector.scalar_tensor_tensor(
                    out=Ft[:, :gf], in0=tt[:, :gf], scalar=1.5,
                    in1=s2ap, op0=Alu.subtract, op1=Alu.mult)
            # Order DVE so the next group's w16 (data-ready early) runs
            # before this F' (which waits on the ACT chain).
            if g + 1 < NG and "winst" in st[g + 1]:
                import bass_rust as _br
                finst.ins.add_nosync_dependencies_from(
                    _br.InstructionNameOrderedSet([st[g + 1]["winst"].ins.name]))
            pss = []
            for k in range(gsz):
                ps = psum.tile([P, 2 * P], f32, tag=f"ps{k}")
                for c in range(NCHUNK):
                    sl = slice(k * FREE + c * P, k * FREE + (c + 1) * P)
                    # psum cols 0:128 = F'^T N, cols 128:256 = F'^T w
                    nc.tensor.matmul(ps[:], Ft[:, sl], nw[:, :, sl],
                                     start=(c == 0), stop=(c == NCHUNK - 1))
                pss.append(ps)
            st[g]["pss"] = pss

        def emit_diag(g):
            # Deferred: diag waits on PE; emitting it late keeps the DVE
            # queue from stalling behind it.
            b0, gsz = groups[g]
            for k in range(gsz):
                b = b0 + k
                ps = st[g]["pss"][k]
                scr = work.tile([P, 2 * P], f32, tag=f"scr{k}")
                # Scol[:,b] = sum_j ps[:,j]*mask[:,j] (diag picks -N, -w)
                nc.vector.scalar_tensor_tensor(
                    out=scr[:], in0=ps[:], scalar=0.0, in1=mask_t[:],
                    op0=Alu.add, op1=Alu.mult,
                    accum_out=Scol[:, b:b + 1])

        emit_load(0)
        nc.sync.dma_start(out=mask_t[:], in_=mask_in.ap())
        emit_load(1)
        emit_w(0)
        emit_act(0)
        for g in range(NG):
            if g + 1 < NG:
                emit_w(g + 1)
            emit_tail(g)
            if g >= 1:
                emit_diag(g - 1)
            if g + 2 < NG:
                emit_load(g + 2)
            if g + 1 < NG:
                emit_act(g + 1)
        emit_diag(NG - 1)

        # Ship per-partition partials; host does the 128-way sum.
        nc.sync.dma_start(out=out_t.ap(), in_=Scol[:])

    nc.compile()
    return nc


def _get_nc():
    if "nc" not in _GLOBAL:
        _GLOBAL["nc"] = _build()
    return _GLOBAL["nc"]


def _mask_np():
    m = np.zeros((P, 2 * P), dtype=np.float32)
    idx = np.arange(P)
    m[idx, idx] = -1.0          # -(F' * N) = F * N
    m[idx, P + idx] = -1.0      # -(F' * w) = F * w
    return m


GAMMA = 2.0
ALPHA = 0.25
SIZE_POWER = 0.5


def kernel(pred: np.ndarray, target: np.ndarray) -> np.ndarray:
    from concourse import bass_utils

    nc = _get_nc()
    pred = np.ascontiguousarray(np.asarray(pred, dtype=np.float32))
    target = np.ascontiguousarray(np.asarray(target, dtype=np.int32))
    mask = _mask_np()

    in_maps = []
    for i in range(N_CORES):
        sl = slice(i * B_PER_CORE, (i + 1) * B_PER_CORE)
        in_maps.append({
            "pred": np.ascontiguousarray(pred[sl, 0]),
            "target": np.ascontiguousarray(target[sl]),
            "mask": mask,
        })

    res = bass_utils.run_bass_kernel_spmd(
        nc, in_maps, core_ids=list(range(N_CORES)),
        trace=bool(_GLOBAL.get("trace", False)),
        **_GLOBAL.get("run_kwargs", {}),
    )
    _GLOBAL["last_results"] = res

    outs = np.stack([r["out"] for r in res.results], axis=0)  # [8, 128, 8]
    S = outs.astype(np.float64).sum(axis=1).reshape(-1)  # per-sample sum(a*s2*L)
    fg = np.count_nonzero(target.reshape(target.shape[0], -1), axis=1)
    fg = fg.astype(np.float64)
    sw = np.where(fg > 0,
                  np.minimum(100.0 / np.power(np.maximum(fg, 1.0), SIZE_POWER), 10.0),
                  1.0)
    per_sample = (S / HW) * sw
    return np.float32(per_sample.mean())



# revision 3
# speedup vs baseline: 1.5479x; 1.5479x over previous
"""Size-weighted focal loss on 8 Trainium2 NeuronCores — v5.

Math (per element, x = logit, t in {0,1}):
  w  = x*(1-2t)         so (1-pt) = sigmoid(w) = u
  L  = -log(pt) = softplus(w) = -ln(1-u)
  a  = 0.75 - 0.5*t     (alpha_t with ALPHA=0.25)
  elem = a * u^2 * L

Host packs w = bf16_rne(x) sign-flipped by t, with t stowed in the bf16
LSB (<=1ulp perturbation). Device input is 4MB/core instead of 16MB —
the baseline's DMA bottleneck — and the strided hi16-XOR DVE pass
disappears.

Device (per core, 8 samples, phase-ordered so each ACT table loads once):
  u    = Sigmoid(w)            [ACT pass 1, table sigmoid_and_others]
  Lv   = Ln(1 - u)  = -L       [ACT pass 2, table natural_log]
  tm   = (w&1) - 1.5 = t - 1.5 [DVE ts, int AND then float SUB]
  g    = tm * u                [DVE tt]
  F    = g * u = (t-1.5)*u^2   [DVE tt]
  PE per sample: psum[128,128] += Lv_chunk^T @ F_chunk  (16 chunks)
  diag extract with mask M[i,i] = 0.5:
    Scol[:,b] = 0.5*sum_diag = sum(a*u^2*L) partials per partition-slot
  (elem = a*u^2*L = 0.5*F*Lv since a = -0.5*(t-1.5), L = -Lv)

Host: fg_b = count_nonzero(target_b); mean_b( (S_b/HW) * sw(fg_b) ).
"""

import numpy as np
from contextlib import ExitStack

P = 128
B_PER_CORE = 8
N_CORES = 8
H = 512
W = 512
HW = H * W                 # 262144
FREE = HW // P             # 2048 per sample
NCHUNK = FREE // P         # 16 chunks per sample

_GLOBAL = {}


def _build():
    import concourse.bacc as bacc
    import concourse.tile as tile
    import concourse.mybir as mybir

    f32 = mybir.dt.float32
    bf16 = mybir.dt.bfloat16
    u16 = mybir.dt.uint16
    Alu = mybir.AluOpType
    Act = mybir.ActivationFunctionType

    nc = bacc.Bacc("TRN2", target_bir_lowering=False, debug=False,
                   num_devices=N_CORES)

    wp_in = nc.dram_tensor("wp", (B_PER_CORE, P, FREE), bf16, kind="ExternalInput")
    mask_in = nc.dram_tensor("mask", (P, P), f32, kind="ExternalInput")
    out_t = nc.dram_tensor("out", (P, B_PER_CORE), f32, kind="ExternalOutput")

    w_v = wp_in.ap()

    with ExitStack() as ctx:
        tc = ctx.enter_context(tile.TileContext(nc))
        singles = ctx.enter_context(tc.tile_pool(name="singles", bufs=1))
        tmpool = ctx.enter_context(tc.tile_pool(name="tmpool", bufs=4))
        gpool = ctx.enter_context(tc.tile_pool(name="gpool", bufs=3))
        fpool = ctx.enter_context(tc.tile_pool(name="fpool", bufs=8))
        lvpool = ctx.enter_context(tc.tile_pool(name="lvpool", bufs=3))
        scrpool = ctx.enter_context(tc.tile_pool(name="scrpool", bufs=2))
        psum = ctx.enter_context(tc.tile_pool(name="psum", bufs=8, space="PSUM"))

        mask_t = singles.tile([P, P], f32)
        Scol = singles.tile([P, B_PER_CORE], f32)
        wt = singles.tile([P, B_PER_CORE * FREE], bf16)   # packed w, all samples
        ut = singles.tile([P, B_PER_CORE * FREE], bf16)   # sigmoid(w)

        def sl(b):
            return slice(b * FREE, (b + 1) * FREE)

        # DMA: per-sample loads first (ring priority), mask after.
        for b in range(B_PER_CORE):
            nc.sync.dma_start(out=wt[:, sl(b)], in_=w_v[b])
        nc.sync.dma_start(out=mask_t[:], in_=mask_in.ap())

        # ACT phase 1: all sigmoids back-to-back -> one table load.
        for b in range(B_PER_CORE):
            nc.scalar.activation(ut[:, sl(b)], wt[:, sl(b)], Act.Sigmoid)

        # DVE: tm = (w&1) - 1.5 (= t - 1.5); then g = tm*u, F = g*u.
        # tm's only need the DMA, g/F wait on ACT; emit 4 tm's up front,
        # interleave the rest so the 4-buf pool never stalls the queue.
        wu = wt[:].bitcast(u16)
        tms = [None] * B_PER_CORE
        fts = [None] * B_PER_CORE

        def emit_tm(b):
            tm = tmpool.tile([P, FREE], u16, tag="tm")
            nc.vector.tensor_scalar(
                out=tm[:], in0=wu[:, sl(b)], scalar1=1, scalar2=0,
                op0=Alu.bitwise_and, op1=Alu.bitwise_or)
            tms[b] = tm

        def emit_gf(b):
            g = gpool.tile([P, FREE], bf16, tag="g")
            nc.vector.scalar_tensor_tensor(
                out=g[:], in0=tms[b][:], scalar=1.5, in1=ut[:, sl(b)],
                op0=Alu.subtract, op1=Alu.mult)
            ft = fpool.tile([P, FREE], bf16, tag="ft")
            nc.vector.tensor_tensor(
                out=ft[:], in0=g[:], in1=ut[:, sl(b)], op=Alu.mult)
            fts[b] = ft

        for b in range(4):
            emit_tm(b)
        for b in range(B_PER_CORE):
            emit_gf(b)
            if b + 4 < B_PER_CORE:
                emit_tm(b + 4)

        # ACT phase 2 (Ln, second table load) + PE per sample.
        # Ln split in halves so PE starts on the first half sooner.
        pss = [None] * B_PER_CORE
        for b in range(B_PER_CORE):
            lv = lvpool.tile([P, FREE], bf16, tag="lv")
            h = FREE // 2
            nc.scalar.activation(lv[:, :h], ut[:, sl(b)][:, :h], Act.Ln,
                                 scale=-1.0, bias=1.0)
            nc.scalar.activation(lv[:, h:], ut[:, sl(b)][:, h:], Act.Ln,
                                 scale=-1.0, bias=1.0)
            ps = psum.tile([P, P], f32, tag="ps")
            for c in range(NCHUNK):
                cs = slice(c * P, (c + 1) * P)
                nc.tensor.matmul(ps[:], lv[:, cs], fts[b][:, cs],
                                 start=(c == 0), stop=(c == NCHUNK - 1))
            pss[b] = ps

        # Diag extraction (end of DVE queue; each waits on its PE chain).
        for b in range(B_PER_CORE):
            scr = scrpool.tile([P, P], f32, tag="scr")
            nc.vector.scalar_tensor_tensor(
                out=scr[:], in0=pss[b][:], scalar=0.0, in1=mask_t[:],
                op0=Alu.add, op1=Alu.mult,
                accum_out=Scol[:, b:b + 1])

        nc.sync.dma_start(out=out_t.ap(), in_=Scol[:])

    nc.compile()
    return nc


def _get_nc():
    if "nc" not in _GLOBAL:
        _GLOBAL["nc"] = _build()
    return _GLOBAL["nc"]


def _mask_np():
    m = np.zeros((P, P), dtype=np.float32)
    idx = np.arange(P)
    m[idx, idx] = 0.5          # elem = 0.5 * F * Lv
    return m


GAMMA = 2.0
ALPHA = 0.25
SIZE_POWER = 0.5


def _pack_w(pred: np.ndarray, target: np.ndarray) -> np.ndarray:
    """w = bf16_rne(pred)*(1-2t) with t in the LSB; [64, P, FREE] bf16."""
    import ml_dtypes

    x = np.ascontiguousarray(pred[:, 0])
    t = (target > 0).astype(np.uint16)
    bits = x.view(np.uint32)
    hi = ((bits + np.uint32(0x7FFF) + ((bits >> np.uint32(16)) & np.uint32(1)))
          >> np.uint32(16)).astype(np.uint16)
    w16 = ((hi ^ (t << np.uint16(15))) & np.uint16(0xFFFE)) | t
    # [B, 512, 512] -> [B, 128, 2048]: row-major (p q) w -> p (q w), no copy
    return w16.reshape(-1, P, FREE).view(ml_dtypes.bfloat16)


def kernel(pred: np.ndarray, target: np.ndarray) -> np.ndarray:
    from concourse import bass_utils

    nc = _get_nc()
    pred = np.ascontiguousarray(np.asarray(pred, dtype=np.float32))
    target = np.ascontiguousarray(np.asarray(target, dtype=np.int32))
    wv = _pack_w(pred, target)
    mask = _mask_np()

    in_maps = []
    for i in range(N_CORES):
        s = slice(i * B_PER_CORE, (i + 1) * B_PER_CORE)
        in_maps.append({
            "wp": np.ascontiguousarray(wv[s]),
            "mask": mask,
        })

    res = bass_utils.run_bass_kernel_spmd(
        nc, in_maps, core_ids=list(range(N_CORES)),
        trace=bool(_GLOBAL.get("trace", False)),
        **_GLOBAL.get("run_kwargs", {}),
    )
    _GLOBAL["last_results"] = res

    outs = np.stack([r["out"] for r in res.results], axis=0)  # [8, 128, 8]
    S = outs.astype(np.float64).sum(axis=1).reshape(-1)       # per-sample sums
    fg = np.count_nonzero(target.reshape(target.shape[0], -1), axis=1)
    fg = fg.astype(np.float64)
    sw = np.where(fg > 0,
                  np.minimum(100.0 / np.power(np.maximum(fg, 1.0), SIZE_POWER), 10.0),
                  1.0)
    per_sample = (S / HW) * sw
    return np.float32(per_sample.mean())
